# revision 19
# baseline (speedup 1.0000x reference)
"""Binarized linear + BatchNorm (eval) on 8 Trainium2 NeuronCores.

Computes: out = BN(sign(x) @ sign(weight).T)  for
  x [8192, 4096] f32, weight [4096, 4096] f32, BN vectors [4096] f32.

Strategy
--------
Sharding: batch 4-way x out_features 2-way (8 cores). Each core gets a
transposed X shard (contraction dim IN on SBUF partitions), a W shard
pre-tiled into [ot, 128, kt, 128] blocks (16KB contiguous per partition
per block -> efficient DMA), and computes outT [2048(O), 2048(B)] f32
locally. No collectives; the host concatenates the 8 tiles.

Per-core: sign() on the scalar engine straight to fp8e4. sign(x) in
{-1,+1} is exact in fp8 and the PE accumulates in fp32 PSUM, so the
binary matmul in fp8 DoubleRow mode (K=256/matmul, 2x bf16 rate) is
bit-exact. BN folds to out = a*acc + b (a = gamma/sqrt(var+eps),
b = beta - mean*a, computed on-device) applied by the vector engine
during PSUM drain.

Dataflow: X is binarized wave-by-wave (4 batch waves of 512 cols) into a
resident fp8 buffer; W streams once through sign into a resident fp8
buffer during wave 0. DMA rings: X on sync HWDGE, W on scalar HWDGE
(independent FIFOs -> W is not stuck behind X), outputs on gpsimd SWDGE.
"""

import numpy as np
from contextlib import ExitStack

B_FULL, IN, OUT = 8192, 4096, 4096
NB_CORES = 8
BI, OI = 4, 2            # batch x out_features core grid
BS = B_FULL // BI        # 2048 batch per core
OS = OUT // OI           # 2048 out_features per core
KT = IN // 128           # 32 k-tiles of 128
NS = KT // 2             # 16 k256 supertiles (DoubleRow)
OT = OS // 128           # 16 out tiles of 128
NBT = BS // 512          # 4 batch tiles of 512
BN_EPS = 1e-05

_CACHE = {}


def _build_program():
    import concourse.tile as tile
    from concourse import mybir, bacc

    F8 = mybir.dt.float8e4
    F32 = mybir.dt.float32
    Sign = mybir.ActivationFunctionType.Sign
    Sqrt = mybir.ActivationFunctionType.Sqrt
    DR = mybir.MatmulPerfMode.DoubleRow

    nc = bacc.Bacc("TRN2", target_bir_lowering=False, debug=False,
                   num_devices=NB_CORES)
    xt = nc.declare_dram_parameter("xt", [IN, BS], F32, isOutput=False)
    w4 = nc.declare_dram_parameter("w4", [OT, 128, KT, 128], F32, isOutput=False)
    # bnv[:, 0:16]=gamma, 16:32=beta, 32:48=mean, 48:64=var, each [128, OT]
    # with column ot holding elements [ot*128 : (ot+1)*128]
    bnv = nc.declare_dram_parameter("bnv", [128, 4 * OT], F32, isOutput=False)
    o = nc.declare_dram_parameter("o", [OS, BS], F32, isOutput=True)

    with tile.TileContext(nc) as tc:
        with ExitStack() as ctx:
            cons = ctx.enter_context(tc.tile_pool(name="cons", bufs=1))
            xsp = ctx.enter_context(tc.tile_pool(name="xs", bufs=4))
            wsp = ctx.enter_context(tc.tile_pool(name="ws", bufs=3))
            obp = ctx.enter_context(tc.tile_pool(name="ob", bufs=4))
            psp = ctx.enter_context(tc.tile_pool(name="ps", bufs=8, space="PSUM"))

            # ---- BN constants: a = g/sqrt(var+eps), b = beta - mean*a,
            # laid out [128, OT]: column ot = per-partition vector for
            # out-tile ot. One contiguous DMA, math off the critical path.
            bn_sb = cons.tile([128, 4 * OT], F32)
            nc.gpsimd.dma_start(bn_sb[:], bnv[:])
            gs = bn_sb[:, 0:OT]
            bs_ = bn_sb[:, OT:2 * OT]
            ms = bn_sb[:, 2 * OT:3 * OT]
            vs = bn_sb[:, 3 * OT:4 * OT]
            a_sb = cons.tile([128, OT], F32)
            b_sb = cons.tile([128, OT], F32)
            std = cons.tile([128, OT], F32)
            eps = cons.tile([128, 1], F32)

            def bn_compute():
                nc.vector.memset(eps[:], BN_EPS)
                nc.scalar.activation(std[:], vs[:], Sqrt, bias=eps[:, 0:1])
                nc.vector.reciprocal(std[:], std[:])
                nc.vector.tensor_mul(a_sb[:], gs[:], std[:])
                nc.vector.tensor_mul(b_sb[:], ms[:], a_sb[:])
                nc.vector.tensor_sub(b_sb[:], bs_[:], b_sb[:])

            # ---- resident fp8 operands
            xb = cons.tile([128, KT, BS], F8)    # 64KB/partition
            wb = cons.tile([128, KT, OS], F8)    # 64KB/partition

            w_pending = {}

            def w_dma_half(ot, h):
                # [128, 16, 128] f32 half-block; 8KB contig/partition
                ws = wsp.tile([128, 16, 128], F32, tag="w0",
                              name=f"ws_{ot}_{h}")
                nc.sync.dma_start(ws[:], w4[ot, :, h * 16:(h + 1) * 16, :])
                w_pending.setdefault(ot, []).append((h, ws))

            def w_dma(ot):
                w_dma_half(ot, 0)
                w_dma_half(ot, 1)

            def w_sign(ot, only_h=None):
                rest = []
                for h, ws in w_pending.pop(ot):
                    if only_h is not None and h != only_h:
                        rest.append((h, ws))
                        continue
                    nc.scalar.activation(
                        wb[:, h * 16:(h + 1) * 16, ot * 128:(ot + 1) * 128],
                        ws[:], Sign)
                if rest:
                    w_pending[ot] = rest

            def x_load(pair, kt, engine="scalar"):
                # [128, 1024] f32 chunk covering two 512-wide batch tiles
                xs = xsp.tile([128, 1024], F32, tag="x0", name=f"xs_{pair}_{kt}")
                nc.sync.dma_start(xs[:], xt[kt * 128:(kt + 1) * 128,
                                            pair * 1024:(pair + 1) * 1024])
                dst = xb[:, kt, pair * 1024:(pair + 1) * 1024]
                if engine == "scalar":
                    nc.scalar.activation(dst, xs[:], Sign)
                else:
                    # gpsimd 2-op sign keeps the scalar engine free
                    xm = xsp.tile([128, 1024], F32, tag="xm", bufs=2,
                                  name=f"xm_{pair}_{kt}")
                    nc.gpsimd.tensor_scalar(xm[:], xs[:], 0.0, None,
                                            mybir.AluOpType.is_gt)
                    nc.gpsimd.tensor_scalar(dst, xm[:], 2.0, -1.0,
                                            mybir.AluOpType.mult,
                                            mybir.AluOpType.add)

            def do_block(ot, nb):
                acc = psp.tile([128, 512], F32, tag="acc", name=f"acc_{ot}_{nb}")
                for s in range(NS):
                    nc.tensor.matmul(
                        acc[:],
                        wb[:, 2 * s:2 * s + 2, ot * 128:(ot + 1) * 128],
                        xb[:, 2 * s:2 * s + 2, nb * 512:(nb + 1) * 512],
                        start=(s == 0), stop=(s == NS - 1),
                        perf_mode=DR)
                ob = obp.tile([128, 512], F32, tag="ob", name=f"ob_{ot}_{nb}")
                nc.vector.tensor_scalar(
                    ob[:], acc[:],
                    a_sb[:, ot:ot + 1], b_sb[:, ot:ot + 1],
                    mybir.AluOpType.mult, mybir.AluOpType.add)
                nc.gpsimd.dma_start(
                    o[ot * 128:(ot + 1) * 128, nb * 512:(nb + 1) * 512], ob[:])

            # Kickoff, hand-interleaved on the sync ring so the first
            # matmuls (needing wb[ot0] h0 + xb kt0,1) can start ~10us in.
            w_dma_half(0, 0)
            x_load(0, 0)
            x_load(0, 1)
            w_sign(0, only_h=0)
            w_dma_half(0, 1)
            w_dma_half(1, 0)
            x_load(0, 2)
            x_load(0, 3)
            w_sign(0, only_h=1)
            w_dma_half(1, 1)
            w_sign(1)
            # Rest of X wave-pair 0 with W for out-tiles 2..7 trickled in
            # (dma halves and sign offset so the scalar engine never waits)
            w_dma_at = {}
            w_sign_at = {}
            for i, ot in enumerate(range(2, 8)):
                w_dma_at.setdefault(5 + 4 * i, []).append((ot, 0))
                w_dma_at.setdefault(7 + 4 * i, []).append((ot, 1))
                w_sign_at[8 + 4 * i] = ot
            for kt in range(4, KT):
                x_load(0, kt)
                for ot, h in w_dma_at.get(kt, ()):
                    w_dma_half(ot, h)
                if kt in w_sign_at:
                    w_sign(w_sign_at[kt])

            bn_compute()

            # wave 0: batch tiles 0,1 per out-tile; W ot 8..15 and X pair-1
            # ride along (X pair-1 signed on gpsimd; scalar does only W)
            for ot in range(OT):
                if ot + 8 < OT:
                    w_dma(ot + 8)
                    w_sign(ot + 8)
                x_load(1, 2 * ot, engine="gpsimd")
                x_load(1, 2 * ot + 1, engine="gpsimd")
                do_block(ot, 0)
                do_block(ot, 1)

            # wave 1: batch tiles 2,3 from resident wb and xb
            for ot in range(OT):
                do_block(ot, 2)
                do_block(ot, 3)

    nc.compile()
    return nc


def make_in_maps(x, weight, bn_gamma, bn_beta, bn_mean, bn_var):
    xt = [np.ascontiguousarray(x[bi * BS:(bi + 1) * BS, :].T) for bi in range(BI)]
    # W pre-tiling: w4[ot, p, kt, q] = weight[oi*OS + ot*128 + q, kt*128 + p]
    w4 = []
    for oi in range(OI):
        ws = weight[oi * OS:(oi + 1) * OS, :]          # [OS(O), IN]
        t = ws.reshape(OT, 128, KT, 128)               # [ot, q, kt, p]
        w4.append(np.ascontiguousarray(t.transpose(0, 3, 2, 1)))
    # BN vectors relayout: [128, 4*OT], column blocks g|beta|mean|var,
    # bnv[p, blk*OT + t] = vec[t*128 + p]
    bnv = []
    for oi in range(OI):
        sl = slice(oi * OS, (oi + 1) * OS)
        cols = [v[sl].reshape(OT, 128).T for v in
                (bn_gamma, bn_beta, bn_mean, bn_var)]
        bnv.append(np.ascontiguousarray(np.concatenate(cols, axis=1)))
    in_maps = []
    for c in range(NB_CORES):
        bi, oi = divmod(c, OI)
        in_maps.append({"xt": xt[bi], "w4": w4[oi], "bnv": bnv[oi]})
    return in_maps


def kernel(x, weight, bn_gamma, bn_beta, bn_mean, bn_var):
    from concourse.bass_utils import run_bass_kernel_spmd

    x = np.asarray(x, dtype=np.float32)
    weight = np.asarray(weight, dtype=np.float32)
    bn_gamma = np.asarray(bn_gamma, dtype=np.float32)
    bn_beta = np.asarray(bn_beta, dtype=np.float32)
    bn_mean = np.asarray(bn_mean, dtype=np.float32)
    bn_var = np.asarray(bn_var, dtype=np.float32)

    if "nc" not in _CACHE:
        _CACHE["nc"] = _build_program()
    nc = _CACHE["nc"]

    in_maps = make_in_maps(x, weight, bn_gamma, bn_beta, bn_mean, bn_var)

    res = run_bass_kernel_spmd(nc, in_maps, list(range(NB_CORES)))
    _CACHE["last_results"] = res

    out = np.empty((B_FULL, OUT), dtype=np.float32)
    for c in range(NB_CORES):
        bi, oi = divmod(c, OI)
        out[bi * BS:(bi + 1) * BS, oi * OS:(oi + 1) * OS] = res.results[c]["o"].T
    return out


# revision 21
# speedup vs baseline: 2.4934x; 2.4934x over previous
"""Binarized linear + BatchNorm (eval) on 8 Trainium2 NeuronCores.

Computes: out = BN(sign(x) @ sign(weight).T)  for
  x [8192, 4096] f32, weight [4096, 4096] f32, BN vectors [4096] f32.

Strategy
--------
Sharding: batch 4-way x out_features 2-way (8 cores). Each core gets a
transposed X shard (contraction dim IN on SBUF partitions), a W shard
pre-tiled into [ot, 128, kt, 128] blocks (16KB contiguous per partition
per block -> efficient DMA), and computes outT [2048(O), 2048(B)] f32
locally. No collectives; the host concatenates the 8 tiles.

Per-core: sign() on the scalar engine straight to fp8e4. sign(x) in
{-1,+1} is exact in fp8 and the PE accumulates in fp32 PSUM, so the
binary matmul in fp8 DoubleRow mode (K=256/matmul, 2x bf16 rate) is
bit-exact. BN folds to out = a*acc + b (a = gamma/sqrt(var+eps),
b = beta - mean*a, computed on-device) applied by the vector engine
during PSUM drain.

Dataflow: X is binarized wave-by-wave (4 batch waves of 512 cols) into a
resident fp8 buffer; W streams once through sign into a resident fp8
buffer during wave 0. DMA rings: X on sync HWDGE, W on scalar HWDGE
(independent FIFOs -> W is not stuck behind X), outputs on gpsimd SWDGE.
"""

import numpy as np
from contextlib import ExitStack

B_FULL, IN, OUT = 8192, 4096, 4096
NB_CORES = 8
BI, OI = 4, 2            # batch x out_features core grid
BS = B_FULL // BI        # 2048 batch per core
OS = OUT // OI           # 2048 out_features per core
KT = IN // 128           # 32 k-tiles of 128
NS = KT // 2             # 16 k256 supertiles (DoubleRow)
OT = OS // 128           # 16 out tiles of 128
NBT = BS // 512          # 4 batch tiles of 512
BN_EPS = 1e-05

_CACHE = {}


def _build_program():
    import concourse.tile as tile
    from concourse import mybir, bacc

    F8 = mybir.dt.float8e4
    F32 = mybir.dt.float32
    Sign = mybir.ActivationFunctionType.Sign
    Sqrt = mybir.ActivationFunctionType.Sqrt
    DR = mybir.MatmulPerfMode.DoubleRow

    nc = bacc.Bacc("TRN2", target_bir_lowering=False, debug=False,
                   num_devices=NB_CORES)
    xt = nc.declare_dram_parameter("xt", [IN, BS], F32, isOutput=False)
    w4 = nc.declare_dram_parameter("w4", [OT, 128, KT, 128], F32, isOutput=False)
    # bnv[:, 0:16]=gamma, 16:32=beta, 32:48=mean, 48:64=var, each [128, OT]
    # with column ot holding elements [ot*128 : (ot+1)*128]
    bnv = nc.declare_dram_parameter("bnv", [128, 4 * OT], F32, isOutput=False)
    o = nc.declare_dram_parameter("o", [OS, BS], F32, isOutput=True)

    with tile.TileContext(nc) as tc:
        with ExitStack() as ctx:
            cons = ctx.enter_context(tc.tile_pool(name="cons", bufs=1))
            xsp = ctx.enter_context(tc.tile_pool(name="xs", bufs=4))
            wsp = ctx.enter_context(tc.tile_pool(name="ws", bufs=3))
            obp = ctx.enter_context(tc.tile_pool(name="ob", bufs=4))
            psp = ctx.enter_context(tc.tile_pool(name="ps", bufs=8, space="PSUM"))

            # ---- BN constants: a = g/sqrt(var+eps), b = beta - mean*a,
            # laid out [128, OT]: column ot = per-partition vector for
            # out-tile ot. One contiguous DMA, math off the critical path.
            bn_sb = cons.tile([128, 4 * OT], F32)
            nc.gpsimd.dma_start(bn_sb[:], bnv[:])
            gs = bn_sb[:, 0:OT]
            bs_ = bn_sb[:, OT:2 * OT]
            ms = bn_sb[:, 2 * OT:3 * OT]
            vs = bn_sb[:, 3 * OT:4 * OT]
            a_sb = cons.tile([128, OT], F32)
            b_sb = cons.tile([128, OT], F32)
            std = cons.tile([128, OT], F32)
            eps = cons.tile([128, 1], F32)

            def bn_compute():
                nc.vector.memset(eps[:], BN_EPS)
                nc.scalar.activation(std[:], vs[:], Sqrt, bias=eps[:, 0:1])
                nc.vector.reciprocal(std[:], std[:])
                nc.vector.tensor_mul(a_sb[:], gs[:], std[:])
                nc.vector.tensor_mul(b_sb[:], ms[:], a_sb[:])
                nc.vector.tensor_sub(b_sb[:], bs_[:], b_sb[:])

            # ---- resident fp8 operands
            xb = cons.tile([128, KT, BS], F8)    # 64KB/partition
            wb = cons.tile([128, KT, OS], F8)    # 64KB/partition

            w_pending = {}

            def w_dma_half(ot, h):
                # [128, 16, 128] f32 half-block; 8KB contig/partition
                ws = wsp.tile([128, 16, 128], F32, tag="w0",
                              name=f"ws_{ot}_{h}")
                nc.sync.dma_start(ws[:], w4[ot, :, h * 16:(h + 1) * 16, :])
                w_pending.setdefault(ot, []).append((h, ws))

            def w_dma(ot):
                w_dma_half(ot, 0)
                w_dma_half(ot, 1)

            def w_sign(ot, only_h=None):
                rest = []
                for h, ws in w_pending.pop(ot):
                    if only_h is not None and h != only_h:
                        rest.append((h, ws))
                        continue
                    nc.scalar.activation(
                        wb[:, h * 16:(h + 1) * 16, ot * 128:(ot + 1) * 128],
                        ws[:], Sign)
                if rest:
                    w_pending[ot] = rest

            def x_load(pair, kt, engine="scalar"):
                # [128, 1024] f32 chunk covering two 512-wide batch tiles
                xs = xsp.tile([128, 1024], F32, tag="x0", name=f"xs_{pair}_{kt}")
                nc.sync.dma_start(xs[:], xt[kt * 128:(kt + 1) * 128,
                                            pair * 1024:(pair + 1) * 1024])
                dst = xb[:, kt, pair * 1024:(pair + 1) * 1024]
                if engine == "scalar":
                    nc.scalar.activation(dst, xs[:], Sign)
                else:
                    # vector 2-op sign offloads the scalar engine
                    xm = xsp.tile([128, 1024], F32, tag="xm", bufs=2,
                                  name=f"xm_{pair}_{kt}")
                    nc.vector.tensor_scalar(xm[:], xs[:], 0.0, None,
                                            mybir.AluOpType.is_gt)
                    nc.vector.tensor_scalar(dst, xm[:], 2.0, -1.0,
                                            mybir.AluOpType.mult,
                                            mybir.AluOpType.add)

            def do_block(ot, nb):
                acc = psp.tile([128, 512], F32, tag="acc", name=f"acc_{ot}_{nb}")
                for s in range(NS):
                    nc.tensor.matmul(
                        acc[:],
                        wb[:, 2 * s:2 * s + 2, ot * 128:(ot + 1) * 128],
                        xb[:, 2 * s:2 * s + 2, nb * 512:(nb + 1) * 512],
                        start=(s == 0), stop=(s == NS - 1),
                        perf_mode=DR)
                ob = obp.tile([128, 512], F32, tag="ob", name=f"ob_{ot}_{nb}")
                nc.vector.tensor_scalar(
                    ob[:], acc[:],
                    a_sb[:, ot:ot + 1], b_sb[:, ot:ot + 1],
                    mybir.AluOpType.mult, mybir.AluOpType.add)
                nc.gpsimd.dma_start(
                    o[ot * 128:(ot + 1) * 128, nb * 512:(nb + 1) * 512], ob[:])

            # Kickoff, hand-interleaved on the sync ring so the first
            # matmuls (needing wb[ot0] h0 + xb kt0,1) can start ~10us in.
            w_dma_half(0, 0)
            x_load(0, 0)
            x_load(0, 1)
            w_sign(0, only_h=0)
            w_dma_half(0, 1)
            w_dma_half(1, 0)
            x_load(0, 2)
            x_load(0, 3)
            w_sign(0, only_h=1)
            w_dma_half(1, 1)
            w_sign(1)
            # Rest of X wave-pair 0 with W for out-tiles 2..7 trickled in
            # (dma halves and sign offset so the scalar engine never waits)
            w_dma_at = {}
            w_sign_at = {}
            for i, ot in enumerate(range(2, 8)):
                w_dma_at.setdefault(5 + 4 * i, []).append((ot, 0))
                w_dma_at.setdefault(7 + 4 * i, []).append((ot, 1))
                w_sign_at[8 + 4 * i] = ot
            for kt in range(4, KT):
                x_load(0, kt)
                for ot, h in w_dma_at.get(kt, ()):
                    w_dma_half(ot, h)
                if kt in w_sign_at:
                    w_sign(w_sign_at[kt])

            bn_compute()

            # wave 0: batch tiles 0,1 per out-tile; W ot 8..15 and X pair-1
            # ride along (X pair-1 signed on gpsimd; scalar does only W)
            for ot in range(OT):
                if ot + 8 < OT:
                    w_dma(ot + 8)
                    w_sign(ot + 8)
                x_load(1, 2 * ot, engine="scalar")
                x_load(1, 2 * ot + 1, engine="vector")
                do_block(ot, 0)
                do_block(ot, 1)

            # wave 1: batch tiles 2,3 from resident wb and xb
            for ot in range(OT):
                do_block(ot, 2)
                do_block(ot, 3)

    nc.compile()
    return nc


def make_in_maps(x, weight, bn_gamma, bn_beta, bn_mean, bn_var):
    xt = [np.ascontiguousarray(x[bi * BS:(bi + 1) * BS, :].T) for bi in range(BI)]
    # W pre-tiling: w4[ot, p, kt, q] = weight[oi*OS + ot*128 + q, kt*128 + p]
    w4 = []
    for oi in range(OI):
        ws = weight[oi * OS:(oi + 1) * OS, :]          # [OS(O), IN]
        t = ws.reshape(OT, 128, KT, 128)               # [ot, q, kt, p]
        w4.append(np.ascontiguousarray(t.transpose(0, 3, 2, 1)))
    # BN vectors relayout: [128, 4*OT], column blocks g|beta|mean|var,
    # bnv[p, blk*OT + t] = vec[t*128 + p]
    bnv = []
    for oi in range(OI):
        sl = slice(oi * OS, (oi + 1) * OS)
        cols = [v[sl].reshape(OT, 128).T for v in
                (bn_gamma, bn_beta, bn_mean, bn_var)]
        bnv.append(np.ascontiguousarray(np.concatenate(cols, axis=1)))
    in_maps = []
    for c in range(NB_CORES):
        bi, oi = divmod(c, OI)
        in_maps.append({"xt": xt[bi], "w4": w4[oi], "bnv": bnv[oi]})
    return in_maps


def kernel(x, weight, bn_gamma, bn_beta, bn_mean, bn_var):
    from concourse.bass_utils import run_bass_kernel_spmd

    x = np.asarray(x, dtype=np.float32)
    weight = np.asarray(weight, dtype=np.float32)
    bn_gamma = np.asarray(bn_gamma, dtype=np.float32)
    bn_beta = np.asarray(bn_beta, dtype=np.float32)
    bn_mean = np.asarray(bn_mean, dtype=np.float32)
    bn_var = np.asarray(bn_var, dtype=np.float32)

    if "nc" not in _CACHE:
        _CACHE["nc"] = _build_program()
    nc = _CACHE["nc"]

    in_maps = make_in_maps(x, weight, bn_gamma, bn_beta, bn_mean, bn_var)

    res = run_bass_kernel_spmd(nc, in_maps, list(range(NB_CORES)))
    _CACHE["last_results"] = res

    out = np.empty((B_FULL, OUT), dtype=np.float32)
    for c in range(NB_CORES):
        bi, oi = divmod(c, OI)
        out[bi * BS:(bi + 1) * BS, oi * OS:(oi + 1) * OS] = res.results[c]["o"].T
    return out
